# revision 10
# baseline (speedup 1.0000x reference)
"""YOLO-style loss kernel for Trainium2, SPMD over 8 NeuronCores.

Inputs (full): pred_tensor [32768,7,7,30] f32, target_tensor [32768,7,7,30] f32.
Output: np.ndarray shape (5,) f32 = (loss_xy, loss_wh, loss_obj, loss_noobj, loss_class).

Strategy: pure data parallel on batch dim; each core gets 4096 samples
(200704 cells). Host converts to fp16 and regroups channels so every hot
on-chip op is a dense step-1 access (DVE 2x packed mode):
  - pred boxes  [n,10] cell-major as (x0,y0,x1,y1, w0,h0,w1,h1, c0,c1)
  - tgt  boxes  [n,10] cell-major as (x0,y0,w0,h0, x1,y1,w1,h1, c0,c1)
  - classes     [20,n] channel-major per chunk (both tensors)
Per 392-cell chunk: IoU responsibility + five masked squared-diff partial
sums, fused on-chip; squares/copies run on the scalar engine, reciprocal via
the ~1cpe approx custom-DVE op, weighted reductions via stt accum columns.
Each core returns a [128, 20] f32 partial-sum tile (5 losses x 4 chunks);
host reduces and divides by N.
"""

import os
import sys

sys.path.insert(0, "/opt/trn_rl_repo")

import numpy as np

import concourse.bass as bass
import concourse.bacc as bacc
import concourse.tile as tile
from concourse import mybir
from concourse import bass_utils

F32 = mybir.dt.float32
F16 = mybir.dt.float16
ALU = mybir.AluOpType
ACT = mybir.ActivationFunctionType

S = 7
B = 2
C = 20
D = 30
N_FULL = 32768
N_CORES = 8
N_SHARD = N_FULL // N_CORES            # 4096 samples per core
R = N_SHARD * S * S                    # 200704 cells per core
P = 128                                # partitions
RP = R // P                            # 1568 cells per partition
NCK = 392                              # cells per partition per chunk
N_CH = RP // NCK                       # 4 chunks

# channel permutations applied on host (fp16 cast + gather)
PERM_P = [0, 1, 5, 6, 2, 3, 7, 8, 4, 9]   # pred:  x0,y0,x1,y1,w0,h0,w1,h1,c0,c1
PERM_T = [0, 1, 2, 3, 5, 6, 7, 8, 4, 9]   # tgt:   x0,y0,w0,h0,x1,y1,w1,h1,c0,c1


def _ins(ap, pos, step, count):
    """Insert a [step, count] dim at free-dim position `pos` (absolute index
    into the ap list, where index 0 is the partition dim)."""
    new = [list(x) for x in ap.ap]
    new.insert(pos, [step, count])
    return bass.AP(tensor=ap.tensor, offset=ap.offset, ap=new)


def _mk(ap, dims):
    """Rebuild the free dims of `ap` (keeping partition dim + offset) as
    `dims` = list of (step, count)."""
    new = [list(ap.ap[0])] + [[s, c] for s, c in dims]
    return bass.AP(tensor=ap.tensor, offset=ap.offset, ap=new)


def build_program():
    nc = bacc.Bacc("TRN2", target_bir_lowering=False, debug=False)
    n = NCK

    pbox = nc.dram_tensor("pbox", [P, N_CH * n * 10], F16, kind="ExternalInput")
    tbox = nc.dram_tensor("tbox", [P, N_CH * n * 10], F16, kind="ExternalInput")
    pcls = nc.dram_tensor("pcls", [P, N_CH * C * n], F16, kind="ExternalInput")
    tcls = nc.dram_tensor("tcls", [P, N_CH * C * n], F16, kind="ExternalInput")
    out = nc.dram_tensor("out", [P, 5 * N_CH], F32, kind="ExternalOutput")

    pbox_v = pbox.ap().rearrange("p (k n c) -> p k n c", k=N_CH, n=n, c=10)
    tbox_v = tbox.ap().rearrange("p (k n c) -> p k n c", k=N_CH, n=n, c=10)
    pcls_v = pcls.ap().rearrange("p (k c i) -> p k c i", k=N_CH, c=C, i=n)
    tcls_v = tcls.ap().rearrange("p (k c i) -> p k c i", k=N_CH, c=C, i=n)

    with tile.TileContext(nc) as tc:
        with (
            tc.tile_pool(name="raw", bufs=2) as raw,
            tc.tile_pool(name="tmp", bufs=1) as tmp,
            tc.tile_pool(name="persist", bufs=1) as persist,
        ):
            acc = persist.tile([P, 5 * N_CH], F32)

            for k in range(N_CH):
                Pb = raw.tile([P, n, 10], F16, tag="Pb")
                Tb = raw.tile([P, n, 10], F16, tag="Tb")
                Pc = raw.tile([P, C, n], F16, tag="Pc")
                Tc = raw.tile([P, C, n], F16, tag="Tc")
                nc.sync.dma_start(out=Pb, in_=pbox_v[:, k])
                nc.sync.dma_start(out=Tb, in_=tbox_v[:, k])
                nc.sync.dma_start(out=Pc, in_=pcls_v[:, k])
                nc.sync.dma_start(out=Tc, in_=tcls_v[:, k])

                # ---- views ----
                pxy4 = Pb[:, :, 0:4]           # (x0,y0,x1,y1) step1
                pwh4 = Pb[:, :, 4:8]           # (w0,h0,w1,h1) step1
                pc2 = Pb[:, :, 8:10]
                txy0 = Tb[:, :, 0:2]
                twh0 = Tb[:, :, 2:4]
                tc2 = Tb[:, :, 8:10]
                obj_src = Tb[:, :, 8]          # [P,n] step10
                # target (x0,y0,x1,y1): [n][box step4][coord step1]
                txy4v = _ins(Tb[:, :, 0:2], 2, 4, 2)
                # target (w0,h0,w1,h1): same with offset 2
                twh4v = _ins(Tb[:, :, 2:4], 2, 4, 2)

                # ---- IoU stage (coords scaled x7: corners 3.5*wh -+ xy) ----
                # ts+tt instead of scalar_tensor_tensor: stt measures 2 cyc/elem
                # on this HW, ts+tt is ~0.5+0.5
                t1 = tmp.tile([P, n, 4], F16, tag="t1")
                nc.vector.tensor_scalar(t1, pwh4, 3.5, None, op0=ALU.mult)
                nl4 = tmp.tile([P, n, 4], F16, tag="nl4")    # -(7l) both boxes
                r4 = tmp.tile([P, n, 4], F16, tag="r4")      # 7r both boxes
                nc.vector.tensor_tensor(nl4, t1, pxy4, op=ALU.subtract)
                nc.vector.tensor_tensor(r4, t1, pxy4, op=ALU.add)

                t2 = tmp.tile([P, n, 2], F16, tag="t2")
                nc.vector.tensor_scalar(t2, twh0, 3.5, None, op0=ALU.mult)
                nlt2 = tmp.tile([P, n, 2], F16, tag="nlt2")
                rt2 = tmp.tile([P, n, 2], F16, tag="rt2")
                nc.vector.tensor_tensor(nlt2, t2, txy0, op=ALU.subtract)
                nc.vector.tensor_tensor(rt2, t2, txy0, op=ALU.add)
                # broadcast target corners over box dim: [n][box step0][coord step1]
                nlt2b = _ins(nlt2[:, :, :], 2, 0, 2)
                rt2b = _ins(rt2[:, :, :], 2, 0, 2)

                mln4 = tmp.tile([P, n, 4], F16, tag="mln4")
                mr4 = tmp.tile([P, n, 4], F16, tag="mr4")
                nc.vector.tensor_tensor(mln4, nl4, nlt2b, op=ALU.min)
                nc.vector.tensor_tensor(mr4, r4, rt2b, op=ALU.min)
                s4 = nl4  # dead, reuse
                nc.vector.tensor_tensor(s4, mln4, mr4, op=ALU.add)   # 7*(minr-maxl)
                cw4 = r4  # dead, reuse
                nc.vector.tensor_scalar(cw4, s4, 1.0 / 7.0, 0.0, op0=ALU.mult, op1=ALU.max)

                # per-box scalars, box-major [P,2,n]
                inter2 = tmp.tile([P, 2, n], F16, tag="inter2")
                areap2 = tmp.tile([P, 2, n], F16, tag="areap2")
                areat = tmp.tile([P, n], F16, tag="areat")
                # cw x/y lanes: [box step2][cell step4] from cw4 (x0,y0,x1,y1)
                cwx = _mk(cw4[:, :, 0], [(2, 2), (4, n)])
                cwy = _mk(cw4[:, :, 1], [(2, 2), (4, n)])
                nc.vector.tensor_tensor(inter2, cwx, cwy, op=ALU.mult)
                pw2 = _mk(Pb[:, :, 4], [(2, 2), (10, n)])
                ph2 = _mk(Pb[:, :, 5], [(2, 2), (10, n)])
                nc.vector.tensor_tensor(areap2, pw2, ph2, op=ALU.mult)
                nc.vector.tensor_tensor(areat, Tb[:, :, 2], Tb[:, :, 3], op=ALU.mult)

                u2h = tmp.tile([P, 2, n], F16, tag="u2h")
                u2 = tmp.tile([P, 2, n], F32, tag="u2")
                nc.vector.tensor_tensor(u2h, areap2, inter2, op=ALU.subtract)
                areatb = _ins(areat[:, :], 1, 0, 2)          # [box step0][cell step1]
                nc.vector.tensor_tensor(u2, u2h, areatb, op=ALU.add)

                rcp2 = tmp.tile([P, 2, n], F32, tag="rcp2")
                nc.vector.reciprocal_approx_fast(rcp2, u2)
                # fp32 operand would drop the iou multiply to ~1571ns; a scalar
                # engine downcast keeps the DVE op at 2x (~361ns)
                rcp16 = tmp.tile([P, 2, n], F16, tag="rcp16")
                nc.scalar.activation(rcp16, rcp2, ACT.Copy)
                iou2 = tmp.tile([P, 2, n], F16, tag="iou2")
                nc.vector.tensor_tensor(iou2, inter2, rcp16, op=ALU.mult)

                is1 = tmp.tile([P, n], F16, tag="is1")
                riou = tmp.tile([P, n], F16, tag="riou")
                nc.vector.tensor_tensor(is1, iou2[:, 1, :], iou2[:, 0, :], op=ALU.is_gt)
                nc.vector.tensor_tensor(riou, iou2[:, 1, :], iou2[:, 0, :], op=ALU.max)

                # obj compact copy (scalar engine) — keeps resp ops at 2x
                obj_c = tmp.tile([P, n], F16, tag="obj_c")
                nc.scalar.activation(obj_c, obj_src, ACT.Copy)

                resp = tmp.tile([P, 2, n], F16, tag="resp")
                nc.vector.tensor_tensor(resp[:, 1, :], obj_c, is1, op=ALU.mult)
                nc.vector.tensor_tensor(resp[:, 0, :], obj_c, resp[:, 1, :], op=ALU.subtract)

                # (r0,r0,r1,r1) per cell, materialized compact so the xy/wh
                # premask multiplies run at 2x
                resp4m = tmp.tile([P, n, 4], F16, tag="resp4m")
                resp4v = _mk(resp[:, 0, 0], [(1, n), (n, 2), (0, 2)])
                nc.scalar.activation(resp4m, resp4v, ACT.Copy)

                # ---- losses: premask (DVE/gpsimd) + Square-with-accum (ACT).
                # All masks are exactly 0/1, so sum(mask*d^2) == sum((mask*d)^2).
                def sqacc(dm, col, junk):
                    nc.scalar.activation(
                        junk, dm, ACT.Square,
                        accum_out=acc[:, 5 * k + col:5 * k + col + 1],
                    )

                junk4 = mln4   # dead
                junk2 = nlt2   # dead

                # xy
                dxy4 = tmp.tile([P, n, 4], F16, tag="dxy4")
                dm4 = tmp.tile([P, n, 4], F16, tag="dm4")
                nc.vector.tensor_tensor(dxy4, txy4v, pxy4, op=ALU.subtract)
                nc.vector.tensor_tensor(dm4, dxy4, resp4m, op=ALU.mult)
                sqacc(dm4, 0, junk4)

                # wh (sqrt space)
                sp4 = tmp.tile([P, n, 4], F16, tag="sp4")
                st4 = tmp.tile([P, n, 4], F16, tag="st4")
                nc.scalar.activation(sp4, pwh4, ACT.Sqrt)
                nc.scalar.activation(st4, twh4v, ACT.Sqrt)
                # dxy4's last reader is the dm4 multiply (DVE) — same-engine reuse
                dwh4 = dxy4
                dmw4 = mr4  # dead, reuse
                nc.vector.tensor_tensor(dwh4, st4, sp4, op=ALU.subtract)
                nc.vector.tensor_tensor(dmw4, dwh4, resp4m, op=ALU.mult)
                sqacc(dmw4, 1, junk4)

                # obj conf vs responsible-iou, box-major [P,2,n]: the diff is 1x
                # (broadcast riou), the premask is all-compact 2x against resp
                dc2 = tmp.tile([P, 2, n], F16, tag="dc2")
                dmc2 = tmp.tile([P, 2, n], F16, tag="dmc2")
                rioub = _ins(riou[:, :], 1, 0, 2)                 # [boxdup][cell]
                pc2bm = _mk(Pb[:, 0, 8], [(1, 2), (10, n)])       # [box][cell]
                nc.vector.tensor_tensor(dc2, rioub, pc2bm, op=ALU.subtract)
                nc.vector.tensor_tensor(dmc2, dc2, resp, op=ALU.mult)
                sqacc(dmc2, 2, junk2)

                # noobj conf: noobj*(tc-pc)^2 == ((tc*pc)-pc)^2 since tc in {0,1}
                # -> all operands compact, both multiplies 2x, no mask tile
                m2 = tmp.tile([P, n, 2], F16, tag="m2")
                dmn2 = tmp.tile([P, n, 2], F16, tag="dmn2")
                nc.vector.tensor_tensor(m2, tc2, pc2, op=ALU.mult)
                nc.vector.tensor_tensor(dmn2, m2, pc2, op=ALU.subtract)
                sqacc(dmn2, 3, junk2)

                # class (channel-major [P,20,n]); premask split DVE/gpsimd to
                # load the otherwise-idle gpsimd engine
                GC = 16
                dcl = tmp.tile([P, C, n], F16, tag="dcl")
                dmcl = tmp.tile([P, C, n], F16, tag="dmcl")
                nc.vector.tensor_tensor(dcl, Tc, Pc, op=ALU.subtract)
                objbG = _mk(obj_c[:, 0], [(0, GC), (1, n)])
                objbV = _mk(obj_c[:, 0], [(0, C - GC), (1, n)])
                nc.gpsimd.tensor_tensor(dmcl[:, 0:GC, :], dcl[:, 0:GC, :], objbG, op=ALU.mult)
                nc.vector.tensor_tensor(dmcl[:, GC:C, :], dcl[:, GC:C, :], objbV, op=ALU.mult)
                junk20 = dcl  # dead once both premasks have read it
                sqacc(dmcl, 4, junk20)

            nc.sync.dma_start(out=out.ap(), in_=acc)

    nc.compile()
    return nc


_nc_cache = None
LAST_EXEC_NS = None
LAST_RESULT = None


def _get_nc():
    global _nc_cache
    if _nc_cache is None:
        _nc_cache = build_program()
    return _nc_cache


def _prep(full, perm):
    """[N*S*S, 30] f32 -> per-core (box [P, N_CH*n*10], cls [P, N_CH*20*n]) f16."""
    A = np.asarray(full, dtype=np.float32).reshape(N_CORES, P, N_CH, NCK, D)
    A16 = A.astype(np.float16)
    box = np.ascontiguousarray(A16[..., perm]).reshape(N_CORES, P, -1)
    cls_ = np.ascontiguousarray(A16[..., 10:30].transpose(0, 1, 2, 4, 3)).reshape(
        N_CORES, P, -1
    )
    return box, cls_


def kernel(pred_tensor, target_tensor):
    global LAST_EXEC_NS, LAST_RESULT
    pred = np.asarray(pred_tensor).reshape(N_FULL * S * S, D)
    tgt = np.asarray(target_tensor).reshape(N_FULL * S * S, D)

    pb, pc = _prep(pred, PERM_P)
    tb, tc = _prep(tgt, PERM_T)

    in_maps = []
    for i in range(N_CORES):
        in_maps.append({"pbox": pb[i], "tbox": tb[i], "pcls": pc[i], "tcls": tc[i]})

    nc = _get_nc()
    trace = bool(os.environ.get("KERNEL_TRACE"))
    tmpdir = os.environ.get("KERNEL_TRACE_DIR") or None
    res = bass_utils.run_bass_kernel_spmd(
        nc, in_maps, core_ids=list(range(N_CORES)), trace=trace, tmpdir=tmpdir
    )
    LAST_RESULT = res
    if res.exec_time_ns is not None:
        LAST_EXEC_NS = res.exec_time_ns
    total = np.zeros(5, dtype=np.float64)
    for m in res.results:
        total += m["out"].astype(np.float64).sum(axis=0).reshape(N_CH, 5).sum(axis=0)
    losses = (total / float(N_FULL)).astype(np.float32)
    return losses


# revision 11
# speedup vs baseline: 1.3153x; 1.3153x over previous
"""YOLO-style loss kernel for Trainium2, SPMD over 8 NeuronCores.

Inputs (full): pred_tensor [32768,7,7,30] f32, target_tensor [32768,7,7,30] f32.
Output: np.ndarray shape (5,) f32 = (loss_xy, loss_wh, loss_obj, loss_noobj, loss_class).

Strategy: pure data parallel on batch dim; each core gets 4096 samples
(200704 cells). Host converts to fp16 and regroups channels so every hot
on-chip op is a dense step-1 access (DVE 2x packed mode):
  - pred boxes  [n,10] cell-major as (x0,y0,x1,y1, w0,h0,w1,h1, c0,c1)
  - tgt  boxes  [n,10] cell-major as (x0,y0,w0,h0, x1,y1,w1,h1, c0,c1)
  - classes     [20,n] channel-major per chunk (both tensors)
Per 392-cell chunk: IoU responsibility + five masked squared-diff partial
sums, fused on-chip; squares/copies run on the scalar engine, reciprocal via
the ~1cpe approx custom-DVE op, weighted reductions via stt accum columns.
Each core returns a [128, 20] f32 partial-sum tile (5 losses x 4 chunks);
host reduces and divides by N.
"""

import os
import sys

sys.path.insert(0, "/opt/trn_rl_repo")

import numpy as np

import concourse.bass as bass
import concourse.bacc as bacc
import concourse.tile as tile
from concourse import mybir
from concourse import bass_utils

F32 = mybir.dt.float32
F16 = mybir.dt.float16
ALU = mybir.AluOpType
ACT = mybir.ActivationFunctionType

S = 7
B = 2
C = 20
D = 30
N_FULL = 32768
N_CORES = 8
N_SHARD = N_FULL // N_CORES            # 4096 samples per core
R = N_SHARD * S * S                    # 200704 cells per core
P = 128                                # partitions
RP = R // P                            # 1568 cells per partition
NCK = 392                              # cells per partition per chunk
N_CH = RP // NCK                       # 4 chunks

# channel permutations applied on host (fp16 cast + gather)
PERM_P = [0, 1, 5, 6, 2, 3, 7, 8, 4, 9]   # pred:  x0,y0,x1,y1,w0,h0,w1,h1,c0,c1
PERM_T = [0, 1, 2, 3, 5, 6, 7, 8, 4, 9]   # tgt:   x0,y0,w0,h0,x1,y1,w1,h1,c0,c1


def _ins(ap, pos, step, count):
    """Insert a [step, count] dim at free-dim position `pos` (absolute index
    into the ap list, where index 0 is the partition dim)."""
    new = [list(x) for x in ap.ap]
    new.insert(pos, [step, count])
    return bass.AP(tensor=ap.tensor, offset=ap.offset, ap=new)


def _mk(ap, dims):
    """Rebuild the free dims of `ap` (keeping partition dim + offset) as
    `dims` = list of (step, count)."""
    new = [list(ap.ap[0])] + [[s, c] for s, c in dims]
    return bass.AP(tensor=ap.tensor, offset=ap.offset, ap=new)


def build_program():
    nc = bacc.Bacc("TRN2", target_bir_lowering=False, debug=False)
    n = NCK

    pbox = nc.dram_tensor("pbox", [P, N_CH * n * 10], F16, kind="ExternalInput")
    tbox = nc.dram_tensor("tbox", [P, N_CH * n * 10], F16, kind="ExternalInput")
    pcls = nc.dram_tensor("pcls", [P, N_CH * C * n], F16, kind="ExternalInput")
    tcls = nc.dram_tensor("tcls", [P, N_CH * C * n], F16, kind="ExternalInput")
    out = nc.dram_tensor("out", [P, 5 * N_CH], F32, kind="ExternalOutput")

    pbox_v = pbox.ap().rearrange("p (k n c) -> p k n c", k=N_CH, n=n, c=10)
    tbox_v = tbox.ap().rearrange("p (k n c) -> p k n c", k=N_CH, n=n, c=10)
    pcls_v = pcls.ap().rearrange("p (k c i) -> p k c i", k=N_CH, c=C, i=n)
    tcls_v = tcls.ap().rearrange("p (k c i) -> p k c i", k=N_CH, c=C, i=n)

    with tile.TileContext(nc) as tc:
        with (
            tc.tile_pool(name="raw", bufs=2) as raw,
            tc.tile_pool(name="tmp", bufs=1) as tmp,
            tc.tile_pool(name="persist", bufs=1) as persist,
        ):
            acc = persist.tile([P, 5 * N_CH], F32)

            for k in range(N_CH):
                Pb = raw.tile([P, n, 10], F16, tag="Pb")
                Tb = raw.tile([P, n, 10], F16, tag="Tb")
                Pc = raw.tile([P, C, n], F16, tag="Pc")
                Tc = raw.tile([P, C, n], F16, tag="Tc")
                nc.sync.dma_start(out=Pb, in_=pbox_v[:, k])
                nc.sync.dma_start(out=Tb, in_=tbox_v[:, k])
                nc.sync.dma_start(out=Pc, in_=pcls_v[:, k])
                nc.sync.dma_start(out=Tc, in_=tcls_v[:, k])

                # ---- views ----
                pxy4 = Pb[:, :, 0:4]           # (x0,y0,x1,y1) step1
                pwh4 = Pb[:, :, 4:8]           # (w0,h0,w1,h1) step1
                pc2 = Pb[:, :, 8:10]
                txy0 = Tb[:, :, 0:2]
                twh0 = Tb[:, :, 2:4]
                tc2 = Tb[:, :, 8:10]
                obj_src = Tb[:, :, 8]          # [P,n] step10
                # target (x0,y0,x1,y1): [n][box step4][coord step1]
                txy4v = _ins(Tb[:, :, 0:2], 2, 4, 2)
                # target (w0,h0,w1,h1): same with offset 2
                twh4v = _ins(Tb[:, :, 2:4], 2, 4, 2)

                # ---- IoU stage (coords scaled x7: corners 3.5*wh -+ xy) ----
                # ts+tt instead of scalar_tensor_tensor: stt measures 2 cyc/elem
                # on this HW, ts+tt is ~0.5+0.5
                t1 = tmp.tile([P, n, 4], F16, tag="t1")
                nc.vector.tensor_scalar(t1, pwh4, 3.5, None, op0=ALU.mult)
                nl4 = tmp.tile([P, n, 4], F16, tag="nl4")    # -(7l) both boxes
                r4 = tmp.tile([P, n, 4], F16, tag="r4")      # 7r both boxes
                nc.vector.tensor_tensor(nl4, t1, pxy4, op=ALU.subtract)
                nc.vector.tensor_tensor(r4, t1, pxy4, op=ALU.add)

                t2 = tmp.tile([P, n, 2], F16, tag="t2")
                nc.vector.tensor_scalar(t2, twh0, 3.5, None, op0=ALU.mult)
                nlt2 = tmp.tile([P, n, 2], F16, tag="nlt2")
                rt2 = tmp.tile([P, n, 2], F16, tag="rt2")
                nc.vector.tensor_tensor(nlt2, t2, txy0, op=ALU.subtract)
                nc.vector.tensor_tensor(rt2, t2, txy0, op=ALU.add)
                # broadcast target corners over box dim: [n][box step0][coord step1]
                nlt2b = _ins(nlt2[:, :, :], 2, 0, 2)
                rt2b = _ins(rt2[:, :, :], 2, 0, 2)

                mln4 = tmp.tile([P, n, 4], F16, tag="mln4")
                mr4 = tmp.tile([P, n, 4], F16, tag="mr4")
                nc.vector.tensor_tensor(mln4, nl4, nlt2b, op=ALU.min)
                nc.vector.tensor_tensor(mr4, r4, rt2b, op=ALU.min)
                s4 = nl4  # dead, reuse
                nc.vector.tensor_tensor(s4, mln4, mr4, op=ALU.add)   # 7*(minr-maxl)
                cw4 = r4  # dead, reuse
                nc.vector.tensor_scalar(cw4, s4, 1.0 / 7.0, 0.0, op0=ALU.mult, op1=ALU.max)

                # per-box scalars, box-major [P,2,n]
                inter2 = tmp.tile([P, 2, n], F16, tag="inter2")
                areap2 = tmp.tile([P, 2, n], F16, tag="areap2")
                areat = tmp.tile([P, n], F16, tag="areat")
                # cw x/y lanes: [box step2][cell step4] from cw4 (x0,y0,x1,y1)
                cwx = _mk(cw4[:, :, 0], [(2, 2), (4, n)])
                cwy = _mk(cw4[:, :, 1], [(2, 2), (4, n)])
                nc.vector.tensor_tensor(inter2, cwx, cwy, op=ALU.mult)
                pw2 = _mk(Pb[:, :, 4], [(2, 2), (10, n)])
                ph2 = _mk(Pb[:, :, 5], [(2, 2), (10, n)])
                nc.vector.tensor_tensor(areap2, pw2, ph2, op=ALU.mult)
                nc.vector.tensor_tensor(areat, Tb[:, :, 2], Tb[:, :, 3], op=ALU.mult)

                u2h = tmp.tile([P, 2, n], F16, tag="u2h")
                u2 = tmp.tile([P, 2, n], F32, tag="u2")
                nc.vector.tensor_tensor(u2h, areap2, inter2, op=ALU.subtract)
                areatb = _ins(areat[:, :], 1, 0, 2)          # [box step0][cell step1]
                nc.vector.tensor_tensor(u2, u2h, areatb, op=ALU.add)

                rcp2 = tmp.tile([P, 2, n], F32, tag="rcp2")
                nc.vector.reciprocal_approx_fast(rcp2, u2)
                # fp32 operand would drop the iou multiply to ~1571ns; a scalar
                # engine downcast keeps the DVE op at 2x (~361ns)
                rcp16 = tmp.tile([P, 2, n], F16, tag="rcp16")
                nc.scalar.activation(rcp16, rcp2, ACT.Copy)
                iou2 = tmp.tile([P, 2, n], F16, tag="iou2")
                nc.vector.tensor_tensor(iou2, inter2, rcp16, op=ALU.mult)

                is1 = tmp.tile([P, n], F16, tag="is1")
                riou = tmp.tile([P, n], F16, tag="riou")
                nc.vector.tensor_tensor(is1, iou2[:, 1, :], iou2[:, 0, :], op=ALU.is_gt)
                nc.vector.tensor_tensor(riou, iou2[:, 1, :], iou2[:, 0, :], op=ALU.max)

                # obj compact copy (scalar engine) — keeps resp ops at 2x
                obj_c = tmp.tile([P, n], F16, tag="obj_c")
                nc.scalar.activation(obj_c, obj_src, ACT.Copy)

                resp = tmp.tile([P, 2, n], F16, tag="resp")
                nc.vector.tensor_tensor(resp[:, 1, :], obj_c, is1, op=ALU.mult)
                nc.vector.tensor_tensor(resp[:, 0, :], obj_c, resp[:, 1, :], op=ALU.subtract)

                # (r0,r0,r1,r1) per cell, materialized compact so the xy/wh
                # premask multiplies run at 2x
                resp4m = tmp.tile([P, n, 4], F16, tag="resp4m")
                resp4v = _mk(resp[:, 0, 0], [(1, n), (n, 2), (0, 2)])
                nc.scalar.activation(resp4m, resp4v, ACT.Copy)

                # ---- losses: premask (DVE/gpsimd) + Square-with-accum (ACT).
                # All masks are exactly 0/1, so sum(mask*d^2) == sum((mask*d)^2).
                def sqacc(dm, col, junk):
                    nc.scalar.activation(
                        junk, dm, ACT.Square,
                        accum_out=acc[:, 5 * k + col:5 * k + col + 1],
                    )

                junk4 = mln4   # dead
                junk2 = nlt2   # dead

                # xy
                dxy4 = tmp.tile([P, n, 4], F16, tag="dxy4")
                dm4 = tmp.tile([P, n, 4], F16, tag="dm4")
                nc.vector.tensor_tensor(dxy4, txy4v, pxy4, op=ALU.subtract)
                nc.vector.tensor_tensor(dm4, dxy4, resp4m, op=ALU.mult)
                sqacc(dm4, 0, junk4)

                # wh (sqrt space)
                sp4 = tmp.tile([P, n, 4], F16, tag="sp4")
                st4 = tmp.tile([P, n, 4], F16, tag="st4")
                nc.scalar.activation(sp4, pwh4, ACT.Sqrt)
                nc.scalar.activation(st4, twh4v, ACT.Sqrt)
                # dxy4's last reader is the dm4 multiply (DVE) — same-engine reuse
                dwh4 = dxy4
                dmw4 = mr4  # dead, reuse
                nc.vector.tensor_tensor(dwh4, st4, sp4, op=ALU.subtract)
                nc.vector.tensor_tensor(dmw4, dwh4, resp4m, op=ALU.mult)
                sqacc(dmw4, 1, junk4)

                # obj conf vs responsible-iou, box-major [P,2,n]: the diff is 1x
                # (broadcast riou), the premask is all-compact 2x against resp
                dc2 = tmp.tile([P, 2, n], F16, tag="dc2")
                dmc2 = tmp.tile([P, 2, n], F16, tag="dmc2")
                rioub = _ins(riou[:, :], 1, 0, 2)                 # [boxdup][cell]
                pc2bm = _mk(Pb[:, 0, 8], [(1, 2), (10, n)])       # [box][cell]
                nc.vector.tensor_tensor(dc2, rioub, pc2bm, op=ALU.subtract)
                nc.vector.tensor_tensor(dmc2, dc2, resp, op=ALU.mult)
                sqacc(dmc2, 2, junk2)

                # noobj conf: noobj*(tc-pc)^2 == ((tc*pc)-pc)^2 since tc in {0,1}
                # -> all operands compact, both multiplies 2x, no mask tile
                m2 = tmp.tile([P, n, 2], F16, tag="m2")
                dmn2 = tmp.tile([P, n, 2], F16, tag="dmn2")
                nc.vector.tensor_tensor(m2, tc2, pc2, op=ALU.mult)
                nc.vector.tensor_tensor(dmn2, m2, pc2, op=ALU.subtract)
                sqacc(dmn2, 3, junk2)

                # class (channel-major [P,20,n]). The obj premask would be a 1x
                # broadcast multiply (8.3us); instead AND the fp16 diffs against
                # a 0xFFFF/0x0000 mask through int32-reinterpreted views — the
                # pair-packing halves the element count (4.25us).
                ffi = tmp.tile([P, n], mybir.dt.int16, tag="ffi")
                nc.scalar.activation(ffi, obj_src, ACT.Copy, scale=-1.0)  # -1 -> 0xFFFF
                ff32 = ffi.bitcast(mybir.dt.int32)                        # [P, n/2]
                ff32b = _mk(ff32[:, 0], [(0, C), (1, n // 2)])

                dcl = tmp.tile([P, C, n], F16, tag="dcl")
                dmcl = tmp.tile([P, C, n], F16, tag="dmcl")
                nc.vector.tensor_tensor(dcl, Tc, Pc, op=ALU.subtract)
                nc.vector.tensor_tensor(
                    dmcl.bitcast(mybir.dt.int32), dcl.bitcast(mybir.dt.int32), ff32b,
                    op=ALU.bitwise_and,
                )
                junk20 = dcl  # dead once the premask has read it
                sqacc(dmcl, 4, junk20)

            nc.sync.dma_start(out=out.ap(), in_=acc)

    nc.compile()
    return nc


_nc_cache = None
LAST_EXEC_NS = None
LAST_RESULT = None


def _get_nc():
    global _nc_cache
    if _nc_cache is None:
        _nc_cache = build_program()
    return _nc_cache


def _prep(full, perm):
    """[N*S*S, 30] f32 -> per-core (box [P, N_CH*n*10], cls [P, N_CH*20*n]) f16."""
    A = np.asarray(full, dtype=np.float32).reshape(N_CORES, P, N_CH, NCK, D)
    A16 = A.astype(np.float16)
    box = np.ascontiguousarray(A16[..., perm]).reshape(N_CORES, P, -1)
    cls_ = np.ascontiguousarray(A16[..., 10:30].transpose(0, 1, 2, 4, 3)).reshape(
        N_CORES, P, -1
    )
    return box, cls_


def kernel(pred_tensor, target_tensor):
    global LAST_EXEC_NS, LAST_RESULT
    pred = np.asarray(pred_tensor).reshape(N_FULL * S * S, D)
    tgt = np.asarray(target_tensor).reshape(N_FULL * S * S, D)

    pb, pc = _prep(pred, PERM_P)
    tb, tc = _prep(tgt, PERM_T)

    in_maps = []
    for i in range(N_CORES):
        in_maps.append({"pbox": pb[i], "tbox": tb[i], "pcls": pc[i], "tcls": tc[i]})

    nc = _get_nc()
    trace = bool(os.environ.get("KERNEL_TRACE"))
    tmpdir = os.environ.get("KERNEL_TRACE_DIR") or None
    res = bass_utils.run_bass_kernel_spmd(
        nc, in_maps, core_ids=list(range(N_CORES)), trace=trace, tmpdir=tmpdir
    )
    LAST_RESULT = res
    if res.exec_time_ns is not None:
        LAST_EXEC_NS = res.exec_time_ns
    total = np.zeros(5, dtype=np.float64)
    for m in res.results:
        total += m["out"].astype(np.float64).sum(axis=0).reshape(N_CH, 5).sum(axis=0)
    losses = (total / float(N_FULL)).astype(np.float32)
    return losses
